# revision 25
# baseline (speedup 1.0000x reference)
"""Trainium2 Bass kernel for Memorynet — PE-stream-interleaved pipeline.

Half-batch stages (8 tiles each). Per stage i:
  weights+scatter+transpose(i-1), prefetch loads, then the selection of
  stage i with the MLP matmuls of stage i-3 interleaved into the
  DVE-paced gaps so the PE queue never drains (keeps PE DVFS ramped).
S-matmul folds -|p1|^2 via 3 extra split rows (K=24) so PSUM holds -d2.
"""

import sys

sys.path.insert(0, "/opt/trn_rl_repo")

import numpy as np
import ml_dtypes

import concourse.bass as bass
import concourse.bacc as bacc_mod
import concourse.mybir as mybir
from concourse.tile import TileContext
from concourse.bass_utils import run_bass_kernel_spmd

EPS_DIST = 1e-8
EPS_BN = 1e-5
NCORES = 8
BPC = 4
N1, N2, C1, C2 = 2048, 512, 128, 256
CIN, H1, H2 = C1 + C2, 256, 128
NT = N1 // 128
GROUP = 4
NG = NT // GROUP
KR = 24

f32 = mybir.dt.float32
bf16 = mybir.dt.bfloat16
u32 = mybir.dt.uint32
i16 = mybir.dt.int16

AT = mybir.ActivationFunctionType
OP = mybir.AluOpType

SHIFT = 3  # mlp of half h interleaves into selection of half h+SHIFT


def build_bass():
    nc = bacc_mod.Bacc()
    p1e = nc.declare_dram_parameter("p1e", [BPC, KR, N1], bf16, isOutput=False)
    rhs4 = nc.declare_dram_parameter("rhs4", [BPC, KR, N2], bf16, isOutput=False)
    f1T = nc.declare_dram_parameter("f1T", [BPC, C1, N1], bf16, isOutput=False)
    g1s = [
        nc.declare_dram_parameter(f"g1_{b}", [N2, H1], bf16, isOutput=False)
        for b in range(BPC)
    ]
    W1fd = nc.declare_dram_parameter("W1fT", [C1, H1], bf16, isOutput=False)
    W2Td = nc.declare_dram_parameter("W2T", [H1, H2], bf16, isOutput=False)
    sb1d = nc.declare_dram_parameter("sb1", [H1, 2], f32, isOutput=False)
    sb2d = nc.declare_dram_parameter("sb2", [H2, 2], f32, isOutput=False)
    outT = nc.declare_dram_parameter("outT", [BPC, H2, N1], f32, isOutput=True)

    with TileContext(nc) as tc:
        with (
            tc.tile_pool(name="const", bufs=1) as cpool,
            tc.tile_pool(name="batch", bufs=3) as bpool,
            tc.tile_pool(name="sel", bufs=2) as selpool,
            tc.tile_pool(name="wts", bufs=2) as wpool,
            tc.tile_pool(name="gk", bufs=24) as gkpool,
            tc.tile_pool(name="diag", bufs=3) as dpool,
            tc.tile_pool(name="xg", bufs=6) as xpool,
            tc.tile_pool(name="h1p", bufs=4) as h1pool,
            tc.tile_pool(name="ps_s", bufs=4, space="PSUM") as ps_s,
            tc.tile_pool(name="ps_mlp", bufs=2, space="PSUM") as ps_mlp,
        ):
            W1f = cpool.tile([C1, H1], bf16)
            nc.sync.dma_start(out=W1f[:], in_=W1fd[:, :])
            W2T = [cpool.tile([128, H2], bf16, tag=f"w2_{k}", name=f"w2_{k}") for k in range(2)]
            for k in range(2):
                nc.sync.dma_start(out=W2T[k][:], in_=W2Td[128 * k:128 * (k + 1), :])
            sb1 = [cpool.tile([128, 2], f32, tag=f"sb1_{k}", name=f"sb1_{k}") for k in range(2)]
            for k in range(2):
                nc.sync.dma_start(out=sb1[k][:], in_=sb1d[128 * k:128 * (k + 1), :])
            sb2 = cpool.tile([128, 2], f32)
            nc.sync.dma_start(out=sb2[:], in_=sb2d[:, :])

            state = {}
            HT = NT // 2
            HG = NG // 2

            def emit_load(b):
                p1eb = bpool.tile([KR, N1], bf16, tag="p1eb")
                nc.sync.dma_start(out=p1eb[:], in_=p1e[b, :, :])
                rhsb = bpool.tile([KR, N2], bf16, tag="rhsb")
                nc.sync.dma_start(out=rhsb[:], in_=rhs4[b, :, :])
                g1sb = bpool.tile([128, 4, H1], bf16, tag="g1sb")
                nc.sync.dma_start(
                    out=g1sb[:],
                    in_=g1s[b][:, :].rearrange("(c p) d -> p c d", p=128),
                )
                state[("in", b)] = (p1eb, rhsb, g1sb)

            def emit_weights(h):
                maxg, idxg = state.pop(("sel", h))
                negd = wpool.tile([128, HT, 8], f32, tag="negd")
                nc.vector.tensor_scalar(
                    out=negd[:], in0=maxg[:], scalar1=-1.0, scalar2=EPS_DIST,
                    op0=OP.mult, op1=OP.add,
                )
                recd = wpool.tile([128, HT, 8], f32, tag="recd")
                nc.vector.reciprocal(out=recd[:], in_=negd[:])
                Z = wpool.tile([128, HT], f32, tag="Z")
                nc.vector.reduce_sum(
                    out=Z[:], in_=recd[:, :, 0:3], axis=mybir.AxisListType.X
                )
                Zinv = wpool.tile([128, HT], f32, tag="Zinv")
                nc.vector.reciprocal(out=Zinv[:], in_=Z[:])
                wg = wpool.tile([128, HT, 8], f32, tag="wg")
                nc.vector.tensor_tensor(
                    out=wg[:], in0=recd[:],
                    in1=Zinv[:, :, None].to_broadcast([128, HT, 8]),
                    op=OP.mult,
                )
                wbf = wpool.tile([128, HT, 4], bf16, tag="wbf")
                nc.vector.tensor_copy(out=wbf[:, :, 0:3], in_=wg[:, :, 0:3])
                nc.vector.memset(wbf[:, :, 3:4], 0.0)
                idx16 = wpool.tile([128, HT, 4], i16, tag="idx16")
                nc.vector.tensor_copy(out=idx16[:, :, 0:3], in_=idxg[:, :, 0:3])
                nc.vector.memset(idx16[:, :, 3:4], -513)
                nc.vector.tensor_scalar_add(
                    idx16[:, 1::2, :], idx16[:, 1::2, :], 512
                )
                state[("w", h)] = (wbf, idx16)

            def emit_scatter(h):
                b, half = divmod(h, 2)
                wbf, idx16 = state.pop(("w", h))
                ats = []
                for gi in range(HG):
                    g = half * HG + gi
                    Ag = dpool.tile([128, GROUP, N2], bf16, tag="A")
                    atts = []
                    for pair in range(2):
                        tp = GROUP * gi + 2 * pair
                        nc.gpsimd.local_scatter(
                            out_ap=Ag[:, 2 * pair:2 * pair + 2, :].rearrange(
                                "p t n -> p (t n)"
                            ),
                            data_ap=wbf[:, tp:tp + 2, :].rearrange(
                                "p t k -> p (t k)"
                            ),
                            idxs_ap=idx16[:, tp:tp + 2, :].rearrange(
                                "p t k -> p (t k)"
                            ),
                            channels=128,
                            num_elems=2 * N2,
                            num_idxs=8,
                        )
                        for tt in range(2):
                            ATt = gkpool.tile([128, 4, 128], bf16, tag="ATt")
                            nc.sync.dma_start_transpose(
                                out=ATt[:], in_=Ag[:, 2 * pair + tt, :]
                            )
                            atts.append(ATt)
                    f1g = xpool.tile([C1, 512], bf16, tag="f1g")
                    nc.scalar.dma_start(
                        out=f1g[:], in_=f1T[b, :, 512 * g:512 * (g + 1)]
                    )
                    ats.append((atts, f1g))
                state[("at", h)] = ats

            def mlp_gen(h):
                """Generator: yields after each PE matmul so the caller can
                interleave them into selection gaps."""
                b, half = divmod(h, 2)
                _, _, g1sb = state[("in", b)]
                ats = state.pop(("at", h))
                h1s = []
                for gi in range(HG):
                    atts, f1g = ats[gi]
                    h1 = [h1pool.tile([128, 512], bf16, tag=f"h1_{m}", name=f"h1_{m}_{h}_{gi}") for m in range(2)]
                    for m in range(2):
                        l1p = ps_mlp.tile([128, 512], f32, tag="l1p")
                        for tt in range(GROUP):
                            for c in range(4):
                                nc.tensor.matmul(
                                    out=l1p[:, 128 * tt:128 * (tt + 1)],
                                    lhsT=g1sb[:, c, 128 * m:128 * (m + 1)],
                                    rhs=atts[tt][:, c, :],
                                    start=(c == 0),
                                    stop=False,
                                )
                                yield
                            nc.tensor.matmul(
                                out=l1p[:, 128 * tt:128 * (tt + 1)],
                                lhsT=W1f[:, 128 * m:128 * (m + 1)],
                                rhs=f1g[:, 128 * tt:128 * (tt + 1)],
                                start=False,
                                stop=True,
                            )
                            yield
                        nc.scalar.activation(
                            out=h1[m][:],
                            in_=l1p[:],
                            func=AT.Relu,
                            scale=sb1[m][:, 0:1],
                            bias=sb1[m][:, 1:2],
                        )
                    h1s.append(h1)
                for gi in range(HG):
                    g = half * HG + gi
                    h1 = h1s[gi]
                    l2p = ps_mlp.tile([128, 512], f32, tag="l2p")
                    for kk in range(2):
                        nc.tensor.matmul(
                            out=l2p[:],
                            lhsT=W2T[kk][:],
                            rhs=h1[kk][:],
                            start=(kk == 0),
                            stop=(kk == 1),
                        )
                        yield
                    o = xpool.tile([128, 512], f32, tag="osb")
                    nc.scalar.activation(
                        out=o[:],
                        in_=l2p[:],
                        func=AT.Relu,
                        scale=sb2[:, 0:1],
                        bias=sb2[:, 1:2],
                    )
                    nc.sync.dma_start(
                        out=outT[b, :, 512 * g:512 * (g + 1)], in_=o[:]
                    )

            def advance(gen, n):
                if gen is None:
                    return
                for _ in range(n):
                    try:
                        next(gen)
                    except StopIteration:
                        break

            def emit_select(h, gen):
                """Selection of half h with MLP matmuls interleaved."""
                b, half = divmod(h, 2)
                p1eb, rhsb, g1sb = state[("in", b)]
                maxg = selpool.tile([128, HT, 8], f32, tag="maxg")
                idxg = selpool.tile([128, HT, 8], u32, tag="idxg")
                sps = []

                def s_mm(i):
                    t = half * HT + i
                    Sp = ps_s.tile([128, N2], f32, tag="Sp")
                    nc.tensor.matmul(
                        out=Sp[:],
                        lhsT=p1eb[:, 128 * t:128 * (t + 1)],
                        rhs=rhsb[:],
                        start=True,
                        stop=True,
                    )
                    sps.append(Sp)

                for i in range(min(4, HT)):
                    s_mm(i)
                for i in range(HT):
                    nc.vector.max(out=maxg[:, i, :], in_=sps[i][:])
                    nc.vector.max_index(
                        out=idxg[:, i, :], in_max=maxg[:, i, :], in_values=sps[i][:]
                    )
                    advance(gen, 12)
                    if i + 4 < HT:
                        s_mm(i + 4)
                advance(gen, 100)
                state[("sel", h)] = (maxg, idxg)

            NH = 2 * BPC
            emit_load(0)
            for stage in range(NH + SHIFT):
                if 1 <= stage <= NH:
                    emit_weights(stage - 1)
                    emit_scatter(stage - 1)
                nxt = stage + 1
                if nxt < NH and nxt % 2 == 0 and nxt // 2 < BPC:
                    emit_load(nxt // 2)
                gen = None
                if stage >= SHIFT:
                    gen = mlp_gen(stage - SHIFT)
                if stage < NH:
                    emit_select(stage, gen)
                else:
                    advance(gen, 1000)
    nc.compile()
    return nc


_CACHE = {}


def _get_nc():
    if "nc" not in _CACHE:
        _CACHE["nc"] = build_bass()
    return _CACHE["nc"]


def _prep_core(inputs, c):
    sl = slice(BPC * c, BPC * (c + 1))
    p1 = inputs["points_1"][sl]
    p2 = inputs["points_2"][sl]
    f1 = inputs["features_1"][sl]
    f2 = inputs["features_2"][sl]

    def split3(x):
        a = x.astype(ml_dtypes.bfloat16)
        r = x - a.astype(np.float32)
        bb = r.astype(ml_dtypes.bfloat16)
        cc = (r - bb.astype(np.float32)).astype(ml_dtypes.bfloat16)
        return a, bb, cc

    p1T = np.transpose(p1, (0, 2, 1)).astype(np.float32)
    p2T2 = (2.0 * np.transpose(p2, (0, 2, 1))).astype(np.float32)
    p2sq = np.sum(p2.astype(np.float64) ** 2, -1)
    a1, b1_, c1_ = split3(p1T)
    x2, y2, z2 = split3(p2T2)
    s1_, s2_, s3_ = split3((-p2sq).astype(np.float32))
    p1sq = np.sum(p1.astype(np.float64) ** 2, -1)
    q1_, q2_, q3_ = split3((-p1sq).astype(np.float32))
    onesr = np.ones((BPC, 1, N1), ml_dtypes.bfloat16)
    ones2 = np.ones((BPC, 1, N2), ml_dtypes.bfloat16)
    p1e = np.concatenate(
        [a1, a1, b1_, a1, b1_, c1_, onesr, onesr, onesr,
         q1_[:, None, :], q2_[:, None, :], q3_[:, None, :]], axis=1
    )
    rhs4 = np.concatenate(
        [x2, y2, x2, z2, y2, x2,
         s1_[:, None, :], s2_[:, None, :], s3_[:, None, :],
         ones2, ones2, ones2], axis=1
    )
    m = {
        "p1e": np.ascontiguousarray(p1e.astype(ml_dtypes.bfloat16)),
        "rhs4": np.ascontiguousarray(rhs4.astype(ml_dtypes.bfloat16)),
        "f1T": np.ascontiguousarray(
            np.transpose(f1, (0, 2, 1)).astype(ml_dtypes.bfloat16)
        ),
    }
    W1r = inputs["W1"][:, 0:C2]
    W1fT = inputs["W1"][:, C2:].T
    for b in range(BPC):
        g1b = f2[b].astype(np.float32) @ W1r.T.astype(np.float32)
        m[f"g1_{b}"] = np.ascontiguousarray(g1b.astype(ml_dtypes.bfloat16))
    m["W1fT"] = np.ascontiguousarray(W1fT.astype(ml_dtypes.bfloat16))
    s1 = inputs["g1"] / np.sqrt(inputs["v1"] + EPS_BN)
    b1f = (inputs["b1"] - inputs["m1"]) * s1 + inputs["be1"]
    s2 = inputs["g2"] / np.sqrt(inputs["v2"] + EPS_BN)
    b2f = (inputs["b2"] - inputs["m2"]) * s2 + inputs["be2"]
    m["W2T"] = np.ascontiguousarray(inputs["W2"].T.astype(ml_dtypes.bfloat16))
    m["sb1"] = np.ascontiguousarray(np.stack([s1, b1f], -1).astype(np.float32))
    m["sb2"] = np.ascontiguousarray(np.stack([s2, b2f], -1).astype(np.float32))
    return m


def run(inputs, trace=False):
    nc = _get_nc()
    in_maps = [_prep_core(inputs, c) for c in range(NCORES)]
    res = run_bass_kernel_spmd(
        nc, in_maps, core_ids=list(range(NCORES)), trace=trace
    )
    outs = [np.asarray(r["outT"]) for r in res.results]
    full = np.concatenate(outs, 0)
    out = np.ascontiguousarray(np.transpose(full, (0, 2, 1)))
    return out, res


def kernel(**inputs):
    out, _ = run(inputs, trace=False)
    return out


# revision 26
# speedup vs baseline: 1.0277x; 1.0277x over previous
"""Trainium2 Bass kernel for Memorynet — PE-stream-interleaved pipeline.

Half-batch stages (8 tiles each). Per stage i:
  weights+scatter+transpose(i-1), prefetch loads, then the selection of
  stage i with the MLP matmuls of stage i-3 interleaved into the
  DVE-paced gaps so the PE queue never drains (keeps PE DVFS ramped).
S-matmul folds -|p1|^2 via 3 extra split rows (K=24) so PSUM holds -d2.
"""

import sys

sys.path.insert(0, "/opt/trn_rl_repo")

import numpy as np
import ml_dtypes

import concourse.bass as bass
import concourse.bacc as bacc_mod
import concourse.mybir as mybir
from concourse.tile import TileContext
from concourse.bass_utils import run_bass_kernel_spmd

EPS_DIST = 1e-8
EPS_BN = 1e-5
NCORES = 8
BPC = 4
N1, N2, C1, C2 = 2048, 512, 128, 256
CIN, H1, H2 = C1 + C2, 256, 128
NT = N1 // 128
GROUP = 4
NG = NT // GROUP
KR = 24

f32 = mybir.dt.float32
bf16 = mybir.dt.bfloat16
u32 = mybir.dt.uint32
i16 = mybir.dt.int16

AT = mybir.ActivationFunctionType
OP = mybir.AluOpType

SHIFT = 3  # mlp of half h interleaves into selection of half h+SHIFT


def build_bass():
    nc = bacc_mod.Bacc()
    p1e = nc.declare_dram_parameter("p1e", [BPC, KR, N1], bf16, isOutput=False)
    rhs4 = nc.declare_dram_parameter("rhs4", [BPC, KR, N2], bf16, isOutput=False)
    f1T = nc.declare_dram_parameter("f1T", [BPC, C1, N1], bf16, isOutput=False)
    g1s = [
        nc.declare_dram_parameter(f"g1_{b}", [N2, H1], bf16, isOutput=False)
        for b in range(BPC)
    ]
    W1fd = nc.declare_dram_parameter("W1fT", [C1, H1], bf16, isOutput=False)
    W2Td = nc.declare_dram_parameter("W2T", [H1, H2], bf16, isOutput=False)
    sb1d = nc.declare_dram_parameter("sb1", [H1, 2], f32, isOutput=False)
    sb2d = nc.declare_dram_parameter("sb2", [H2, 2], f32, isOutput=False)
    outT = nc.declare_dram_parameter("outT", [BPC, H2, N1], f32, isOutput=True)

    with TileContext(nc) as tc:
        with (
            tc.tile_pool(name="const", bufs=1) as cpool,
            tc.tile_pool(name="batch", bufs=3) as bpool,
            tc.tile_pool(name="sel", bufs=2) as selpool,
            tc.tile_pool(name="wts", bufs=2) as wpool,
            tc.tile_pool(name="gk", bufs=6) as gkpool,
            tc.tile_pool(name="diag", bufs=3) as dpool,
            tc.tile_pool(name="xg", bufs=6) as xpool,
            tc.tile_pool(name="h1p", bufs=4) as h1pool,
            tc.tile_pool(name="ps_s", bufs=4, space="PSUM") as ps_s,
            tc.tile_pool(name="ps_mlp", bufs=2, space="PSUM") as ps_mlp,
        ):
            W1f = cpool.tile([C1, H1], bf16)
            nc.sync.dma_start(out=W1f[:], in_=W1fd[:, :])
            W2T = [cpool.tile([128, H2], bf16, tag=f"w2_{k}", name=f"w2_{k}") for k in range(2)]
            for k in range(2):
                nc.sync.dma_start(out=W2T[k][:], in_=W2Td[128 * k:128 * (k + 1), :])
            sb1 = [cpool.tile([128, 2], f32, tag=f"sb1_{k}", name=f"sb1_{k}") for k in range(2)]
            for k in range(2):
                nc.sync.dma_start(out=sb1[k][:], in_=sb1d[128 * k:128 * (k + 1), :])
            sb2 = cpool.tile([128, 2], f32)
            nc.sync.dma_start(out=sb2[:], in_=sb2d[:, :])

            state = {}
            HT = NT // 2
            HG = NG // 2

            def emit_load(b):
                p1eb = bpool.tile([KR, N1], bf16, tag="p1eb")
                nc.sync.dma_start(out=p1eb[:], in_=p1e[b, :, :])
                rhsb = bpool.tile([KR, N2], bf16, tag="rhsb")
                nc.sync.dma_start(out=rhsb[:], in_=rhs4[b, :, :])
                g1sb = bpool.tile([128, 4, H1], bf16, tag="g1sb")
                nc.sync.dma_start(
                    out=g1sb[:],
                    in_=g1s[b][:, :].rearrange("(c p) d -> p c d", p=128),
                )
                state[("in", b)] = (p1eb, rhsb, g1sb)

            def emit_weights(h):
                maxg, idxg = state.pop(("sel", h))
                negd = wpool.tile([128, HT, 8], f32, tag="negd")
                nc.vector.tensor_scalar(
                    out=negd[:], in0=maxg[:], scalar1=-1.0, scalar2=EPS_DIST,
                    op0=OP.mult, op1=OP.add,
                )
                recd = wpool.tile([128, HT, 8], f32, tag="recd")
                nc.vector.reciprocal(out=recd[:], in_=negd[:])
                Z = wpool.tile([128, HT], f32, tag="Z")
                nc.vector.reduce_sum(
                    out=Z[:], in_=recd[:, :, 0:3], axis=mybir.AxisListType.X
                )
                Zinv = wpool.tile([128, HT], f32, tag="Zinv")
                nc.vector.reciprocal(out=Zinv[:], in_=Z[:])
                wg = wpool.tile([128, HT, 8], f32, tag="wg")
                nc.vector.tensor_tensor(
                    out=wg[:], in0=recd[:],
                    in1=Zinv[:, :, None].to_broadcast([128, HT, 8]),
                    op=OP.mult,
                )
                wbf = wpool.tile([128, HT, 4], bf16, tag="wbf")
                nc.vector.tensor_copy(out=wbf[:, :, 0:3], in_=wg[:, :, 0:3])
                nc.vector.memset(wbf[:, :, 3:4], 0.0)
                idx16 = wpool.tile([128, HT, 4], i16, tag="idx16")
                nc.vector.tensor_copy(out=idx16[:, :, 0:3], in_=idxg[:, :, 0:3])
                nc.vector.memset(idx16[:, :, 3:4], -513)
                nc.vector.tensor_scalar_add(
                    idx16[:, 1::2, :], idx16[:, 1::2, :], 512
                )
                state[("w", h)] = (wbf, idx16)

            def emit_scatter(h):
                b, half = divmod(h, 2)
                wbf, idx16 = state.pop(("w", h))
                ats = []
                for gi in range(HG):
                    g = half * HG + gi
                    Ag = dpool.tile([128, GROUP, N2], bf16, tag="A")
                    for pair in range(2):
                        tp = GROUP * gi + 2 * pair
                        nc.gpsimd.local_scatter(
                            out_ap=Ag[:, 2 * pair:2 * pair + 2, :].rearrange(
                                "p t n -> p (t n)"
                            ),
                            data_ap=wbf[:, tp:tp + 2, :].rearrange(
                                "p t k -> p (t k)"
                            ),
                            idxs_ap=idx16[:, tp:tp + 2, :].rearrange(
                                "p t k -> p (t k)"
                            ),
                            channels=128,
                            num_elems=2 * N2,
                            num_idxs=8,
                        )
                    ATt = gkpool.tile([128, 16, 128], bf16, tag="ATt")
                    nc.sync.dma_start_transpose(out=ATt[:], in_=Ag[:])
                    f1g = xpool.tile([C1, 512], bf16, tag="f1g")
                    nc.scalar.dma_start(
                        out=f1g[:], in_=f1T[b, :, 512 * g:512 * (g + 1)]
                    )
                    ats.append((ATt, f1g))
                state[("at", h)] = ats

            def mlp_gen(h):
                """Generator: yields after each PE matmul so the caller can
                interleave them into selection gaps."""
                b, half = divmod(h, 2)
                _, _, g1sb = state[("in", b)]
                ats = state.pop(("at", h))
                h1s = []
                for gi in range(HG):
                    ATt, f1g = ats[gi]
                    ATv = ATt[:].rearrange("p (t c) r -> p c t r", c=4)
                    h1 = [h1pool.tile([128, 512], bf16, tag=f"h1_{m}", name=f"h1_{m}_{h}_{gi}") for m in range(2)]
                    for m in range(2):
                        l1p = ps_mlp.tile([128, 512], f32, tag="l1p")
                        for c in range(4):
                            nc.tensor.matmul(
                                out=l1p[:],
                                lhsT=g1sb[:, c, 128 * m:128 * (m + 1)],
                                rhs=ATv[:, c],
                                start=(c == 0),
                                stop=False,
                            )
                            yield
                        nc.tensor.matmul(
                            out=l1p[:],
                            lhsT=W1f[:, 128 * m:128 * (m + 1)],
                            rhs=f1g[:],
                            start=False,
                            stop=True,
                        )
                        nc.scalar.activation(
                            out=h1[m][:],
                            in_=l1p[:],
                            func=AT.Relu,
                            scale=sb1[m][:, 0:1],
                            bias=sb1[m][:, 1:2],
                        )
                        yield
                    h1s.append(h1)
                for gi in range(HG):
                    g = half * HG + gi
                    h1 = h1s[gi]
                    l2p = ps_mlp.tile([128, 512], f32, tag="l2p")
                    for kk in range(2):
                        nc.tensor.matmul(
                            out=l2p[:],
                            lhsT=W2T[kk][:],
                            rhs=h1[kk][:],
                            start=(kk == 0),
                            stop=(kk == 1),
                        )
                        yield
                    o = xpool.tile([128, 512], f32, tag="osb")
                    nc.scalar.activation(
                        out=o[:],
                        in_=l2p[:],
                        func=AT.Relu,
                        scale=sb2[:, 0:1],
                        bias=sb2[:, 1:2],
                    )
                    nc.scalar.dma_start(
                        out=outT[b, :, 512 * g:512 * (g + 1)], in_=o[:]
                    )

            def advance(gen, n):
                if gen is None:
                    return
                for _ in range(n):
                    try:
                        next(gen)
                    except StopIteration:
                        break

            def emit_select(h, gen):
                """Selection of half h with MLP matmuls interleaved."""
                b, half = divmod(h, 2)
                p1eb, rhsb, g1sb = state[("in", b)]
                maxg = selpool.tile([128, HT, 8], f32, tag="maxg")
                idxg = selpool.tile([128, HT, 8], u32, tag="idxg")
                sps = []

                def s_mm(i):
                    t = half * HT + i
                    Sp = ps_s.tile([128, N2], f32, tag="Sp")
                    nc.tensor.matmul(
                        out=Sp[:],
                        lhsT=p1eb[:, 128 * t:128 * (t + 1)],
                        rhs=rhsb[:],
                        start=True,
                        stop=True,
                    )
                    sps.append(Sp)

                for i in range(min(4, HT)):
                    s_mm(i)
                for i in range(HT):
                    nc.vector.max(out=maxg[:, i, :], in_=sps[i][:])
                    nc.vector.max_index(
                        out=idxg[:, i, :], in_max=maxg[:, i, :], in_values=sps[i][:]
                    )
                    advance(gen, 4)
                    if i + 4 < HT:
                        s_mm(i + 4)
                advance(gen, 100)
                state[("sel", h)] = (maxg, idxg)

            NH = 2 * BPC
            emit_load(0)
            for stage in range(NH + SHIFT):
                if 1 <= stage <= NH:
                    emit_weights(stage - 1)
                    emit_scatter(stage - 1)
                nxt = stage + 1
                if nxt < NH and nxt % 2 == 0 and nxt // 2 < BPC:
                    emit_load(nxt // 2)
                gen = None
                if stage >= SHIFT:
                    gen = mlp_gen(stage - SHIFT)
                if stage < NH:
                    emit_select(stage, gen)
                else:
                    advance(gen, 1000)
    nc.compile()
    return nc


_CACHE = {}


def _get_nc():
    if "nc" not in _CACHE:
        _CACHE["nc"] = build_bass()
    return _CACHE["nc"]


def _prep_core(inputs, c):
    sl = slice(BPC * c, BPC * (c + 1))
    p1 = inputs["points_1"][sl]
    p2 = inputs["points_2"][sl]
    f1 = inputs["features_1"][sl]
    f2 = inputs["features_2"][sl]

    def split3(x):
        a = x.astype(ml_dtypes.bfloat16)
        r = x - a.astype(np.float32)
        bb = r.astype(ml_dtypes.bfloat16)
        cc = (r - bb.astype(np.float32)).astype(ml_dtypes.bfloat16)
        return a, bb, cc

    p1T = np.transpose(p1, (0, 2, 1)).astype(np.float32)
    p2T2 = (2.0 * np.transpose(p2, (0, 2, 1))).astype(np.float32)
    p2sq = np.sum(p2.astype(np.float64) ** 2, -1)
    a1, b1_, c1_ = split3(p1T)
    x2, y2, z2 = split3(p2T2)
    s1_, s2_, s3_ = split3((-p2sq).astype(np.float32))
    p1sq = np.sum(p1.astype(np.float64) ** 2, -1)
    q1_, q2_, q3_ = split3((-p1sq).astype(np.float32))
    onesr = np.ones((BPC, 1, N1), ml_dtypes.bfloat16)
    ones2 = np.ones((BPC, 1, N2), ml_dtypes.bfloat16)
    p1e = np.concatenate(
        [a1, a1, b1_, a1, b1_, c1_, onesr, onesr, onesr,
         q1_[:, None, :], q2_[:, None, :], q3_[:, None, :]], axis=1
    )
    rhs4 = np.concatenate(
        [x2, y2, x2, z2, y2, x2,
         s1_[:, None, :], s2_[:, None, :], s3_[:, None, :],
         ones2, ones2, ones2], axis=1
    )
    m = {
        "p1e": np.ascontiguousarray(p1e.astype(ml_dtypes.bfloat16)),
        "rhs4": np.ascontiguousarray(rhs4.astype(ml_dtypes.bfloat16)),
        "f1T": np.ascontiguousarray(
            np.transpose(f1, (0, 2, 1)).astype(ml_dtypes.bfloat16)
        ),
    }
    W1r = inputs["W1"][:, 0:C2]
    W1fT = inputs["W1"][:, C2:].T
    for b in range(BPC):
        g1b = f2[b].astype(np.float32) @ W1r.T.astype(np.float32)
        m[f"g1_{b}"] = np.ascontiguousarray(g1b.astype(ml_dtypes.bfloat16))
    m["W1fT"] = np.ascontiguousarray(W1fT.astype(ml_dtypes.bfloat16))
    s1 = inputs["g1"] / np.sqrt(inputs["v1"] + EPS_BN)
    b1f = (inputs["b1"] - inputs["m1"]) * s1 + inputs["be1"]
    s2 = inputs["g2"] / np.sqrt(inputs["v2"] + EPS_BN)
    b2f = (inputs["b2"] - inputs["m2"]) * s2 + inputs["be2"]
    m["W2T"] = np.ascontiguousarray(inputs["W2"].T.astype(ml_dtypes.bfloat16))
    m["sb1"] = np.ascontiguousarray(np.stack([s1, b1f], -1).astype(np.float32))
    m["sb2"] = np.ascontiguousarray(np.stack([s2, b2f], -1).astype(np.float32))
    return m


def run(inputs, trace=False):
    nc = _get_nc()
    in_maps = [_prep_core(inputs, c) for c in range(NCORES)]
    res = run_bass_kernel_spmd(
        nc, in_maps, core_ids=list(range(NCORES)), trace=trace
    )
    outs = [np.asarray(r["outT"]) for r in res.results]
    full = np.concatenate(outs, 0)
    out = np.ascontiguousarray(np.transpose(full, (0, 2, 1)))
    return out, res


def kernel(**inputs):
    out, _ = run(inputs, trace=False)
    return out


# revision 27
# speedup vs baseline: 1.2213x; 1.1883x over previous
"""Trainium2 Bass kernel for Memorynet — PE-stream-interleaved pipeline.

Half-batch stages (8 tiles each). Per stage i:
  weights+scatter+transpose(i-1), prefetch loads, then the selection of
  stage i with the MLP matmuls of stage i-3 interleaved into the
  DVE-paced gaps so the PE queue never drains (keeps PE DVFS ramped).
S-matmul folds -|p1|^2 via 3 extra split rows (K=24) so PSUM holds -d2.
"""

import sys

sys.path.insert(0, "/opt/trn_rl_repo")

import numpy as np
import ml_dtypes

import concourse.bass as bass
import concourse.bacc as bacc_mod
import concourse.mybir as mybir
from concourse.tile import TileContext
from concourse.bass_utils import run_bass_kernel_spmd

EPS_DIST = 1e-8
EPS_BN = 1e-5
NCORES = 8
BPC = 4
N1, N2, C1, C2 = 2048, 512, 128, 256
CIN, H1, H2 = C1 + C2, 256, 128
NT = N1 // 128
GROUP = 4
NG = NT // GROUP
KR = 24

f32 = mybir.dt.float32
bf16 = mybir.dt.bfloat16
u32 = mybir.dt.uint32
i16 = mybir.dt.int16

AT = mybir.ActivationFunctionType
OP = mybir.AluOpType

SHIFT = 3  # mlp of half h interleaves into selection of half h+SHIFT


def build_bass():
    nc = bacc_mod.Bacc()
    p1e = nc.declare_dram_parameter("p1e", [BPC, KR, N1], bf16, isOutput=False)
    rhs4 = nc.declare_dram_parameter("rhs4", [BPC, KR, N2], bf16, isOutput=False)
    f1T = nc.declare_dram_parameter("f1T", [BPC, C1, N1], bf16, isOutput=False)
    g1s = [
        nc.declare_dram_parameter(f"g1_{b}", [N2, H1], bf16, isOutput=False)
        for b in range(BPC)
    ]
    W1fd = nc.declare_dram_parameter("W1fT", [C1, H1], bf16, isOutput=False)
    W2Td = nc.declare_dram_parameter("W2T", [H1, H2], bf16, isOutput=False)
    sb1d = nc.declare_dram_parameter("sb1", [H1, 2], f32, isOutput=False)
    sb2d = nc.declare_dram_parameter("sb2", [H2, 2], f32, isOutput=False)
    outT = nc.declare_dram_parameter("outT", [BPC, H2, N1], f32, isOutput=True)

    with TileContext(nc) as tc:
        with (
            tc.tile_pool(name="const", bufs=1) as cpool,
            tc.tile_pool(name="batch", bufs=3) as bpool,
            tc.tile_pool(name="sel", bufs=2) as selpool,
            tc.tile_pool(name="wts", bufs=2) as wpool,
            tc.tile_pool(name="gk", bufs=6) as gkpool,
            tc.tile_pool(name="diag", bufs=3) as dpool,
            tc.tile_pool(name="xg", bufs=6) as xpool,
            tc.tile_pool(name="h1p", bufs=4) as h1pool,
            tc.tile_pool(name="ps_s", bufs=4, space="PSUM") as ps_s,
            tc.tile_pool(name="ps_mlp", bufs=2, space="PSUM") as ps_mlp,
        ):
            W1f = cpool.tile([C1, H1], bf16)
            nc.sync.dma_start(out=W1f[:], in_=W1fd[:, :])
            W2T = [cpool.tile([128, H2], bf16, tag=f"w2_{k}", name=f"w2_{k}") for k in range(2)]
            for k in range(2):
                nc.sync.dma_start(out=W2T[k][:], in_=W2Td[128 * k:128 * (k + 1), :])
            sb1 = [cpool.tile([128, 2], f32, tag=f"sb1_{k}", name=f"sb1_{k}") for k in range(2)]
            for k in range(2):
                nc.sync.dma_start(out=sb1[k][:], in_=sb1d[128 * k:128 * (k + 1), :])
            sb2 = cpool.tile([128, 2], f32)
            nc.sync.dma_start(out=sb2[:], in_=sb2d[:, :])

            state = {}
            HT = NT // 2
            HG = NG // 2

            def emit_load(b):
                p1eb = bpool.tile([KR, N1], bf16, tag="p1eb")
                nc.sync.dma_start(out=p1eb[:], in_=p1e[b, :, :])
                rhsb = bpool.tile([KR, N2], bf16, tag="rhsb")
                nc.sync.dma_start(out=rhsb[:], in_=rhs4[b, :, :])
                g1sb = bpool.tile([128, 4, H1], bf16, tag="g1sb")
                nc.sync.dma_start(
                    out=g1sb[:],
                    in_=g1s[b][:, :].rearrange("(c p) d -> p c d", p=128),
                )
                state[("in", b)] = (p1eb, rhsb, g1sb)

            def emit_weights(h):
                maxg, idxg = state.pop(("sel", h))
                negd = wpool.tile([128, HT, 8], f32, tag="negd")
                nc.vector.tensor_scalar(
                    out=negd[:], in0=maxg[:], scalar1=-1.0, scalar2=EPS_DIST,
                    op0=OP.mult, op1=OP.add,
                )
                recd = wpool.tile([128, HT, 8], f32, tag="recd")
                nc.vector.reciprocal(out=recd[:], in_=negd[:])
                Z = wpool.tile([128, HT], f32, tag="Z")
                nc.vector.reduce_sum(
                    out=Z[:], in_=recd[:, :, 0:3], axis=mybir.AxisListType.X
                )
                Zinv = wpool.tile([128, HT], f32, tag="Zinv")
                nc.vector.reciprocal(out=Zinv[:], in_=Z[:])
                wg = wpool.tile([128, HT, 8], f32, tag="wg")
                nc.vector.tensor_tensor(
                    out=wg[:], in0=recd[:],
                    in1=Zinv[:, :, None].to_broadcast([128, HT, 8]),
                    op=OP.mult,
                )
                wbf = wpool.tile([128, HT, 4], bf16, tag="wbf")
                nc.vector.tensor_copy(out=wbf[:, :, 0:3], in_=wg[:, :, 0:3])
                nc.vector.memset(wbf[:, :, 3:4], 0.0)
                idx16 = wpool.tile([128, HT, 4], i16, tag="idx16")
                nc.vector.tensor_copy(out=idx16[:, :, 0:3], in_=idxg[:, :, 0:3])
                nc.vector.memset(idx16[:, :, 3:4], -513)
                nc.vector.tensor_scalar_add(
                    idx16[:, 1::2, :], idx16[:, 1::2, :], 512
                )
                state[("w", h)] = (wbf, idx16)

            def emit_scatter(h):
                b, half = divmod(h, 2)
                wbf, idx16 = state.pop(("w", h))
                ats = []
                for gi in range(HG):
                    g = half * HG + gi
                    Ag = dpool.tile([128, GROUP, N2], bf16, tag="A")
                    for pair in range(2):
                        tp = GROUP * gi + 2 * pair
                        nc.gpsimd.local_scatter(
                            out_ap=Ag[:, 2 * pair:2 * pair + 2, :].rearrange(
                                "p t n -> p (t n)"
                            ),
                            data_ap=wbf[:, tp:tp + 2, :].rearrange(
                                "p t k -> p (t k)"
                            ),
                            idxs_ap=idx16[:, tp:tp + 2, :].rearrange(
                                "p t k -> p (t k)"
                            ),
                            channels=128,
                            num_elems=2 * N2,
                            num_idxs=8,
                        )
                    ATt = gkpool.tile([128, 16, 128], bf16, tag="ATt")
                    nc.sync.dma_start_transpose(out=ATt[:], in_=Ag[:])
                    f1g = xpool.tile([C1, 512], bf16, tag="f1g")
                    nc.scalar.dma_start(
                        out=f1g[:], in_=f1T[b, :, 512 * g:512 * (g + 1)]
                    )
                    ats.append((ATt, f1g))
                state[("at", h)] = ats

            def mlp_gen(h):
                """Generator: yields after each PE matmul so the caller can
                interleave them into selection gaps."""
                b, half = divmod(h, 2)
                _, _, g1sb = state[("in", b)]
                ats = state.pop(("at", h))
                h1s = []
                for gi in range(HG):
                    ATt, f1g = ats[gi]
                    ATv = ATt[:].rearrange("p (t c) r -> p c t r", c=4)
                    h1 = [h1pool.tile([128, 512], bf16, tag=f"h1_{m}", name=f"h1_{m}_{h}_{gi}") for m in range(2)]
                    for m in range(2):
                        l1p = ps_mlp.tile([128, 512], f32, tag="l1p")
                        for c in range(4):
                            nc.tensor.matmul(
                                out=l1p[:],
                                lhsT=g1sb[:, c, 128 * m:128 * (m + 1)],
                                rhs=ATv[:, c],
                                start=(c == 0),
                                stop=False,
                            )
                            yield
                        nc.tensor.matmul(
                            out=l1p[:],
                            lhsT=W1f[:, 128 * m:128 * (m + 1)],
                            rhs=f1g[:],
                            start=False,
                            stop=True,
                        )
                        nc.scalar.activation(
                            out=h1[m][:],
                            in_=l1p[:],
                            func=AT.Relu,
                            scale=sb1[m][:, 0:1],
                            bias=sb1[m][:, 1:2],
                        )
                        yield
                    h1s.append(h1)
                for gi in range(HG):
                    g = half * HG + gi
                    h1 = h1s[gi]
                    l2p = ps_mlp.tile([128, 512], f32, tag="l2p")
                    for kk in range(2):
                        nc.tensor.matmul(
                            out=l2p[:],
                            lhsT=W2T[kk][:],
                            rhs=h1[kk][:],
                            start=(kk == 0),
                            stop=(kk == 1),
                        )
                        yield
                    o = xpool.tile([128, 512], f32, tag="osb")
                    nc.scalar.activation(
                        out=o[:],
                        in_=l2p[:],
                        func=AT.Relu,
                        scale=sb2[:, 0:1],
                        bias=sb2[:, 1:2],
                    )
                    nc.sync.dma_start(
                        out=outT[b, :, 512 * g:512 * (g + 1)], in_=o[:]
                    )

            def advance(gen, n):
                if gen is None:
                    return
                for _ in range(n):
                    try:
                        next(gen)
                    except StopIteration:
                        break

            def emit_select(h, gen):
                """Selection of half h with MLP matmuls interleaved."""
                b, half = divmod(h, 2)
                p1eb, rhsb, g1sb = state[("in", b)]
                maxg = selpool.tile([128, HT, 8], f32, tag="maxg")
                idxg = selpool.tile([128, HT, 8], u32, tag="idxg")
                sps = []

                def s_mm(i):
                    t = half * HT + i
                    Sp = ps_s.tile([128, N2], f32, tag="Sp")
                    nc.tensor.matmul(
                        out=Sp[:],
                        lhsT=p1eb[:, 128 * t:128 * (t + 1)],
                        rhs=rhsb[:],
                        start=True,
                        stop=True,
                    )
                    sps.append(Sp)

                for i in range(min(4, HT)):
                    s_mm(i)
                for i in range(HT):
                    nc.vector.max(out=maxg[:, i, :], in_=sps[i][:])
                    nc.vector.max_index(
                        out=idxg[:, i, :], in_max=maxg[:, i, :], in_values=sps[i][:]
                    )
                    advance(gen, 4)
                    if i + 4 < HT:
                        s_mm(i + 4)
                advance(gen, 100)
                state[("sel", h)] = (maxg, idxg)

            NH = 2 * BPC
            emit_load(0)
            for stage in range(NH + SHIFT):
                if 1 <= stage <= NH:
                    emit_weights(stage - 1)
                    emit_scatter(stage - 1)
                nxt = stage + 1
                if nxt < NH and nxt % 2 == 0 and nxt // 2 < BPC:
                    emit_load(nxt // 2)
                gen = None
                if stage >= SHIFT:
                    gen = mlp_gen(stage - SHIFT)
                if stage < NH:
                    emit_select(stage, gen)
                else:
                    advance(gen, 1000)
    nc.compile()
    return nc


_CACHE = {}


def _get_nc():
    if "nc" not in _CACHE:
        _CACHE["nc"] = build_bass()
    return _CACHE["nc"]


def _prep_core(inputs, c):
    sl = slice(BPC * c, BPC * (c + 1))
    p1 = inputs["points_1"][sl]
    p2 = inputs["points_2"][sl]
    f1 = inputs["features_1"][sl]
    f2 = inputs["features_2"][sl]

    def split3(x):
        a = x.astype(ml_dtypes.bfloat16)
        r = x - a.astype(np.float32)
        bb = r.astype(ml_dtypes.bfloat16)
        cc = (r - bb.astype(np.float32)).astype(ml_dtypes.bfloat16)
        return a, bb, cc

    p1T = np.transpose(p1, (0, 2, 1)).astype(np.float32)
    p2T2 = (2.0 * np.transpose(p2, (0, 2, 1))).astype(np.float32)
    p2sq = np.sum(p2.astype(np.float64) ** 2, -1)
    a1, b1_, c1_ = split3(p1T)
    x2, y2, z2 = split3(p2T2)
    s1_, s2_, s3_ = split3((-p2sq).astype(np.float32))
    p1sq = np.sum(p1.astype(np.float64) ** 2, -1)
    q1_, q2_, q3_ = split3((-p1sq).astype(np.float32))
    onesr = np.ones((BPC, 1, N1), ml_dtypes.bfloat16)
    ones2 = np.ones((BPC, 1, N2), ml_dtypes.bfloat16)
    p1e = np.concatenate(
        [a1, a1, b1_, a1, b1_, c1_, onesr, onesr, onesr,
         q1_[:, None, :], q2_[:, None, :], q3_[:, None, :]], axis=1
    )
    rhs4 = np.concatenate(
        [x2, y2, x2, z2, y2, x2,
         s1_[:, None, :], s2_[:, None, :], s3_[:, None, :],
         ones2, ones2, ones2], axis=1
    )
    m = {
        "p1e": np.ascontiguousarray(p1e.astype(ml_dtypes.bfloat16)),
        "rhs4": np.ascontiguousarray(rhs4.astype(ml_dtypes.bfloat16)),
        "f1T": np.ascontiguousarray(
            np.transpose(f1, (0, 2, 1)).astype(ml_dtypes.bfloat16)
        ),
    }
    W1r = inputs["W1"][:, 0:C2]
    W1fT = inputs["W1"][:, C2:].T
    for b in range(BPC):
        g1b = f2[b].astype(np.float32) @ W1r.T.astype(np.float32)
        m[f"g1_{b}"] = np.ascontiguousarray(g1b.astype(ml_dtypes.bfloat16))
    m["W1fT"] = np.ascontiguousarray(W1fT.astype(ml_dtypes.bfloat16))
    s1 = inputs["g1"] / np.sqrt(inputs["v1"] + EPS_BN)
    b1f = (inputs["b1"] - inputs["m1"]) * s1 + inputs["be1"]
    s2 = inputs["g2"] / np.sqrt(inputs["v2"] + EPS_BN)
    b2f = (inputs["b2"] - inputs["m2"]) * s2 + inputs["be2"]
    m["W2T"] = np.ascontiguousarray(inputs["W2"].T.astype(ml_dtypes.bfloat16))
    m["sb1"] = np.ascontiguousarray(np.stack([s1, b1f], -1).astype(np.float32))
    m["sb2"] = np.ascontiguousarray(np.stack([s2, b2f], -1).astype(np.float32))
    return m


def run(inputs, trace=False):
    nc = _get_nc()
    in_maps = [_prep_core(inputs, c) for c in range(NCORES)]
    res = run_bass_kernel_spmd(
        nc, in_maps, core_ids=list(range(NCORES)), trace=trace
    )
    outs = [np.asarray(r["outT"]) for r in res.results]
    full = np.concatenate(outs, 0)
    out = np.ascontiguousarray(np.transpose(full, (0, 2, 1)))
    return out, res


def kernel(**inputs):
    out, _ = run(inputs, trace=False)
    return out
